# revision 6
# baseline (speedup 1.0000x reference)
"""Tensor-parallel attention kernel for 8 Trainium2 NeuronCores (bf16).

Shards the 32 attention heads across 8 cores (4 heads each). All matmuls run
in bf16 (f32 PSUM accumulation); end-to-end max-rel error vs the f32
reference is ~5e-3 (gate is 2e-2).

Per-core structure:
  Phase 1  QKV projection, weight-stationary: out[o, tok] = sum_d w[d,o] x[d,tok].
           Q/K land directly in the [head-dim, token] layout attention needs
           (no PE transposes); RoPE is applied on PSUM evict using
           host-transposed cos/sin [64, T] (even head dims in partitions
           0-63, odd in 64-127 via a host-side row permutation of wq/wk).
           V is transposed to [token, e] via the DMA XBAR (16-bit transpose).
  Phase 2  Attention per (b, head): one key-tile sweep jt=0..15; per jt up to
           4 query-macro score matmuls share the stationary k block; causal
           mask is a vector add into PSUM; exp on the scalar engine writes
           bf16 prob tiles; softmax denominators accumulate on gpsimd
           (no PE ones-matmuls); AV matmuls trail one jt for pipelining.
  Phase 3  Output projection from SBUF-resident per-head ot tiles.

Each core computes its heads' contribution through wo; the host sums the 8
partial outputs.
"""

import math
import sys

sys.path.insert(0, "/opt/trn_rl_repo")

import numpy as np
import ml_dtypes

import concourse.bacc as bacc
import concourse.bass as bass
import concourse.mybir as mybir
import concourse.tile as tile

F32 = mybir.dt.float32
F32R = mybir.dt.float32r
BF16 = mybir.dt.bfloat16
AF = mybir.ActivationFunctionType
ALU = mybir.AluOpType

HEAD_DIM = 128
NEG = -1.0e30


class Cfg:
    def __init__(self, B=2, S=2048, D=4096, H_PER=4, n_cores=8):
        self.B, self.S, self.D, self.H_PER = B, S, D, H_PER
        self.n_cores = n_cores
        self.T = B * S                    # total tokens (batch-major)
        self.O = H_PER * HEAD_DIM         # per-core projection width
        self.DC = D // 128                # contraction chunks
        self.TT = self.T // 128           # 128-token tiles
        self.TM = self.T // 512           # 512-token macro tiles
        self.NJT = S // 128               # key tiles per batch


def build(cfg: Cfg) -> bacc.Bacc:
    B, S, D, T, O = cfg.B, cfg.S, cfg.D, cfg.T, cfg.O
    H_PER, DC, TT, TM, NJT = cfg.H_PER, cfg.DC, cfg.TT, cfg.TM, cfg.NJT
    scale = 1.0 / math.sqrt(HEAD_DIM)

    nc = bacc.Bacc(None, target_bir_lowering=False)

    xt = nc.dram_tensor("xt", [TM, 128, DC, 512], BF16, kind="ExternalInput")
    wqt = nc.dram_tensor("wqt", [128, DC, O], BF16, kind="ExternalInput")
    wkt = nc.dram_tensor("wkt", [128, DC, O], BF16, kind="ExternalInput")
    wvt = nc.dram_tensor("wvt", [128, DC, O], BF16, kind="ExternalInput")
    wot = nc.dram_tensor("wot", [128, H_PER, D], BF16, kind="ExternalInput")
    cost = nc.dram_tensor("cost", [64, T], BF16, kind="ExternalInput")
    sint = nc.dram_tensor("sint", [64, T], BF16, kind="ExternalInput")
    maskt = nc.dram_tensor("maskt", [4 * 128, 512], F32, kind="ExternalInput")
    onest = nc.dram_tensor("onest", [128, 128], F32R, kind="ExternalInput")
    out = nc.dram_tensor("out", [T, D], F32, kind="ExternalOutput")

    # DRAM scratch: q/k in [head-dim, token] layout; v per head [tok%128, jt, e]
    q_s = nc.dram_tensor("q_s", [O, T], BF16)
    k_s = nc.dram_tensor("k_s", [O, T], BF16)
    v_s = nc.dram_tensor("v_s", [H_PER, 128, B * NJT, 128], BF16)

    with tile.TileContext(nc) as tc:
        # ================= Phase 1: QKV projection (weight-stationary) =====
        with tc.tile_pool(name="w1", bufs=1) as wp, \
             tc.tile_pool(name="sb1", bufs=1) as sb, \
             tc.tile_pool(name="ps1", bufs=1, space="PSUM") as ps:
            cos_sb = wp.tile([64, T], BF16, name="cos_sb")
            sin_sb = wp.tile([64, T], BF16, name="sin_sb")
            wq_sb = wp.tile([128, DC, O], BF16, name="wq_sb")
            wk_sb = wp.tile([128, DC, O], BF16, name="wk_sb")
            wv_sb = wp.tile([128, DC, O], BF16, name="wv_sb")
            for wsb, wdram in ((wq_sb, wqt), (wk_sb, wkt), (wv_sb, wvt)):
                for c4 in range(4):
                    nc.scalar.dma_start(out=wsb[:, c4 * 8:(c4 + 1) * 8, :],
                                        in_=wdram[:, c4 * 8:(c4 + 1) * 8, :])

            for tm in range(TM):
                xq = [sb.tile([128, DC // 4, 512], BF16, name="xs", tag=f"xs{q}",
                              bufs=2) for q in range(4)]
                for q in range(4):
                    nc.sync.dma_start(out=xq[q][:],
                                      in_=xt[tm][:, q * 8:(q + 1) * 8, :])
                    if tm == 0 and q == 0:
                        nc.sync.dma_start(out=cos_sb[:], in_=cost[:])
                        nc.sync.dma_start(out=sin_sb[:], in_=sint[:])
                cs = cos_sb[:, tm * 512:(tm + 1) * 512]
                sn = sin_sb[:, tm * 512:(tm + 1) * 512]
                for ob in range(12):
                    wsb = (wq_sb, wk_sb, wv_sb)[ob // 4]
                    col = ob % 4
                    pp = ps.tile([128, 512], F32, name="pp", tag=f"p{ob % 8}",
                                 bufs=1)
                    for d in range(DC):
                        nc.tensor.matmul(pp[:], wsb[:, d, col * 128:(col + 1) * 128],
                                         xq[d // 8][:, d % 8, :], start=(d == 0),
                                         stop=(d == DC - 1))
                    if ob < 8:
                        # RoPE on evict: partitions 0-63 even dims, 64-127 odd
                        dst = q_s if ob < 4 else k_s
                        pe, po = pp[0:64, :], pp[64:128, :]
                        rot = sb.tile([128, 512], BF16, name="rot", tag="rot", bufs=3)
                        tec = sb.tile([64, 512], F32, name="tec", tag="tec", bufs=2)
                        tos = sb.tile([64, 512], F32, name="tos", tag="tos", bufs=2)
                        tes = sb.tile([64, 512], F32, name="tes", tag="tes", bufs=2)
                        toc = sb.tile([64, 512], F32, name="toc", tag="toc", bufs=2)
                        nc.vector.tensor_tensor(tec[:], pe, cs, ALU.mult)
                        nc.vector.tensor_tensor(tos[:], po, sn, ALU.mult)
                        nc.vector.tensor_tensor(tes[:], pe, sn, ALU.mult)
                        nc.vector.tensor_tensor(toc[:], po, cs, ALU.mult)
                        nc.gpsimd.tensor_tensor(rot[0:64, :], tec[:], tos[:],
                                                ALU.subtract)
                        nc.gpsimd.tensor_tensor(rot[64:128, :], tes[:], toc[:],
                                                ALU.add)
                        nc.sync.dma_start(
                            out=dst[col * 128:(col + 1) * 128,
                                    tm * 512:(tm + 1) * 512],
                            in_=rot[:])
                    else:
                        # V: evict bf16 then transpose 128x128 blocks via XBAR
                        vt = sb.tile([128, 512], BF16, name="vt", tag="vt", bufs=2)
                        nc.scalar.activation(vt[:], pp[:], AF.Copy)
                        vstg = sb.tile([128, 4, 128], BF16, name="vstg",
                                       tag="vstg", bufs=2)
                        for blk in range(4):
                            nc.scalar.dma_start(out=vstg[:, blk, :],
                                                in_=vt[:, blk * 128:(blk + 1) * 128],
                                                transpose=True)
                        nc.scalar.dma_start(out=v_s[col][:, tm * 4:tm * 4 + 4, :],
                                            in_=vstg[:])

        # ================= Phase 2+3: attention + output projection ========
        with tc.tile_pool(name="otp", bufs=1) as otp:
            ot_sb = [otp.tile([128, T], BF16, name=f"ot{h}") for h in range(H_PER)]
            wo_sb = otp.tile([128, H_PER, D], BF16, name="wo_sb")
            with tc.tile_pool(name="asb", bufs=1) as asb, \
                 tc.tile_pool(name="aps", bufs=1, space="PSUM") as aps, \
                 tc.tile_pool(name="sb3", bufs=1) as sb3, \
                 tc.tile_pool(name="ps3", bufs=1, space="PSUM") as ps3:
                mask_sb = asb.tile([128, 4, 512], F32, name="mask_sb")
                ones_sb = asb.tile([128, 128], F32R, name="ones_sb")
                nc.sync.dma_start(out=mask_sb[:],
                                  in_=maskt[:].rearrange("(m p) i -> p m i", p=128))
                nc.sync.dma_start(out=ones_sb[:], in_=onest[:])
                def load_kqv(h):
                    kt = asb.tile([128, T], BF16, name="kt", tag="kt", bufs=2)
                    qt = asb.tile([128, T], BF16, name="qt", tag="qt", bufs=2)
                    vv = asb.tile([128, B * NJT, 128], BF16, name="vv", tag="vv",
                                  bufs=2)
                    nc.sync.dma_start(out=kt[:], in_=k_s[h * 128:(h + 1) * 128, :])
                    nc.sync.dma_start(out=qt[:], in_=q_s[h * 128:(h + 1) * 128, :])
                    nc.sync.dma_start(out=vv[:], in_=v_s[h])
                    return kt, qt, vv

                def half_sweep(b, h, kt, qt, vv, im_lo):
                    ims_all = (im_lo, im_lo + 1)
                    nprs = 2 * (im_lo + 2)
                    pot = {im: aps.tile([128, 512], F32, name="pot",
                                        tag=f"pot{im % 2}", bufs=1)
                           for im in ims_all}
                    ds = {im: asb.tile([128, 512], F32R, name="ds",
                                       tag=f"ds{im % 2}", bufs=2)
                          for im in ims_all}

                    def emit_av(pr, pts):
                        for u in range(2):
                            jt = 2 * pr + u
                            for im in ims_all:
                                if jt < 4 * (im + 1):
                                    nc.tensor.matmul(pot[im][:],
                                                     vv[:, b * NJT + jt, :],
                                                     pts[im][:, u, :],
                                                     start=(jt == 0),
                                                     stop=(jt == 4 * im + 3))

                    prev = None
                    for pr in range(nprs):
                        ims = [im for im in ims_all if pr < 2 * (im + 1)]
                        cur_pst = {}
                        for im in ims:
                            pp = aps.tile([128, 2, 512], F32, name="pst",
                                          tag=f"pst{im % 2}", bufs=1)
                            for u in range(2):
                                jt = 2 * pr + u
                                nc.tensor.matmul(
                                    pp[:, u, :],
                                    kt[:, b * S + jt * 128:b * S + jt * 128 + 128],
                                    qt[:, b * S + im * 512:b * S + (im + 1) * 512],
                                    start=True, stop=True)
                            cur_pst[im] = pp
                        for u in range(2):
                            jt = 2 * pr + u
                            im_d = jt // 4
                            if im_d in ims:
                                nc.vector.tensor_tensor(cur_pst[im_d][:, u, :],
                                                        cur_pst[im_d][:, u, :],
                                                        mask_sb[:, jt % 4, :],
                                                        ALU.add)
                        if prev is not None:
                            emit_av(*prev)
                        cur_pt = {}
                        # exp the masked (diag) tile last: its vector-add dep
                        # shouldn't stall the scalar engine
                        im_d = (2 * pr) // 4
                        order = [im for im in ims if im != im_d] + \
                                ([im_d] if im_d in ims else [])
                        for im in order:
                            pt_t = asb.tile([128, 2, 512], BF16, name="pt",
                                            tag=f"pt{im % 2}", bufs=2)
                            nc.scalar.activation(pt_t[:], cur_pst[im][:], AF.Exp,
                                                 scale=scale)
                            eng = nc.gpsimd if im % 2 == 0 else nc.vector
                            if pr == 0:
                                eng.tensor_tensor(ds[im][:], pt_t[:, 0, :],
                                                  pt_t[:, 1, :], ALU.add)
                            else:
                                tmp = asb.tile([128, 512], F32R, name="tmp",
                                               tag=f"tmp{im % 2}", bufs=2)
                                eng.tensor_tensor(tmp[:], pt_t[:, 0, :],
                                                  pt_t[:, 1, :], ALU.add)
                                eng.tensor_tensor(ds[im][:], ds[im][:], tmp[:],
                                                  ALU.add)
                            cur_pt[im] = pt_t
                        prev = (pr, cur_pt)
                    emit_av(*prev)
                    for im in ims_all:
                        pden = aps.tile([128, 2, 512], F32, name="pden",
                                        tag=f"pst{im % 2}", bufs=1)
                        nc.tensor.matmul(pden[:, 0, :], ones_sb[:], ds[im][:],
                                         start=True, stop=True)
                        r = asb.tile([128, 512], F32, name="rs", tag="rs", bufs=2)
                        with nc.allow_low_precision(reason="softmax recip"):
                            nc.vector.reciprocal(r[:], pden[:, 0, :])
                        nc.vector.tensor_tensor(
                            ot_sb[h][:, b * S + im * 512:b * S + (im + 1) * 512],
                            pot[im][:], r[:], ALU.mult)

                def oproj(tts):
                    for tt in tts:
                        f_sb = sb3.tile([128, D], F32, name="f_sb", tag="f_sb",
                                        bufs=3)
                        for ep in range(4):
                            pf = [ps3.tile([128, 512], F32, name="pf",
                                           tag=f"pf{e2}", bufs=1)
                                  for e2 in range(2)]
                            for hh in range(H_PER):
                                for e2 in range(2):
                                    e = 2 * ep + e2
                                    nc.tensor.matmul(
                                        pf[e2][:],
                                        ot_sb[hh][:, tt * 128:(tt + 1) * 128],
                                        wo_sb[:, hh, e * 512:(e + 1) * 512],
                                        start=(hh == 0), stop=(hh == H_PER - 1))
                            for e2 in range(2):
                                e = 2 * ep + e2
                                if e2 == 0:
                                    nc.vector.tensor_copy(
                                        f_sb[:, e * 512:(e + 1) * 512], pf[e2][:])
                                else:
                                    nc.scalar.activation(
                                        f_sb[:, e * 512:(e + 1) * 512], pf[e2][:],
                                        AF.Copy)
                        q = nc.sync if tt % 2 == 0 else nc.scalar
                        q.dma_start(out=out[tt * 128:(tt + 1) * 128, :],
                                    in_=f_sb[:])

                for h in range(H_PER):
                    kt, qt, vv = load_kqv(h)
                    nc.sync.dma_start(out=wo_sb[:, h, :], in_=wot[:, h, :])
                    half_sweep(0, h, kt, qt, vv, 0)
                    half_sweep(0, h, kt, qt, vv, 2)
                for h in range(H_PER):
                    kt, qt, vv = load_kqv(h)
                    half_sweep(1, h, kt, qt, vv, 0)
                    half_sweep(1, h, kt, qt, vv, 2)
                    oproj(range(4 * h, 4 * h + 4))
                oproj(range(16, TT))

    nc.compile()
    return nc


# host-side even/odd permutation of head dims (RoPE becomes partition-half
# elementwise in the [head-dim, token] layout)
_PERM = np.concatenate([np.arange(0, HEAD_DIM, 2), np.arange(1, HEAD_DIM, 2)])


def host_inputs(cfg: Cfg, x, wq, wk, wv, wo, freqs_cos, freqs_sin):
    """Build the 8 per-core input maps from full inputs (numpy f32)."""
    B, S, D, T, O, H_PER = cfg.B, cfg.S, cfg.D, cfg.T, cfg.O, cfg.H_PER
    DC, TM = cfg.DC, cfg.TM
    bf = ml_dtypes.bfloat16

    xb = np.ascontiguousarray(x.reshape(T, D)).astype(bf)
    # xt[tm, p, c, t] = x.T[c*128+p, tm*512+t]
    xt = np.ascontiguousarray(
        xb.T.reshape(DC, 128, TM, 512).transpose(2, 1, 0, 3))

    cosT = np.ascontiguousarray(np.tile(freqs_cos, (B, 1)).T).astype(bf)
    sinT = np.ascontiguousarray(np.tile(freqs_sin, (B, 1)).T).astype(bf)

    # maskt[k*128 + j, i] = 0 if k*128 + j <= i else NEG
    j_idx = np.arange(4 * 128)[:, None]
    i_idx = np.arange(512)[None, :]
    maskt = np.where(j_idx <= i_idx, 0.0, NEG).astype(np.float32)

    def wtile(w_rows):  # [D, O] -> [128, DC, O]
        return np.ascontiguousarray(
            w_rows.T.reshape(DC, 128, O).transpose(1, 0, 2)).astype(bf)

    in_maps = []
    for c in range(cfg.n_cores):
        rows = []
        for hh in range(H_PER):
            base = (c * H_PER + hh) * HEAD_DIM
            rows.append(base + _PERM)
        prows = np.concatenate(rows)                     # permuted rows for q/k
        nrows = np.arange(c * O, (c + 1) * O)            # natural rows for v
        wot_t = np.ascontiguousarray(
            wo[:, nrows].T.reshape(H_PER, 128, D).transpose(1, 0, 2)).astype(bf)
        in_maps.append({
            "xt": xt,
            "wqt": wtile(wq[prows]),
            "wkt": wtile(wk[prows]),
            "wvt": wtile(wv[nrows]),
            "wot": wot_t,
            "cost": cosT, "sint": sinT, "maskt": maskt,
            "onest": np.ones((128, 128), np.float32),
        })
    return in_maps


_CACHE = {}


def kernel(x, wq, wk, wv, wo, freqs_cos, freqs_sin, mask=None, start_pos=0):
    cfg = Cfg()
    x = np.asarray(x, dtype=np.float32)
    in_maps = host_inputs(cfg, x, np.asarray(wq, np.float32),
                          np.asarray(wk, np.float32),
                          np.asarray(wv, np.float32),
                          np.asarray(wo, np.float32),
                          np.asarray(freqs_cos, np.float32),
                          np.asarray(freqs_sin, np.float32))
    if "nc" not in _CACHE:
        _CACHE["nc"] = build(cfg)
    from concourse.bass_utils import run_bass_kernel_spmd
    res = run_bass_kernel_spmd(_CACHE["nc"], in_maps, core_ids=list(range(cfg.n_cores)))
    acc = res.results[0]["out"].astype(np.float64)
    for c in range(1, cfg.n_cores):
        acc = acc + res.results[c]["out"]
    return acc.astype(np.float32).reshape(cfg.B, cfg.S, cfg.D)


# revision 7
# speedup vs baseline: 1.0241x; 1.0241x over previous
"""Tensor-parallel attention kernel for 8 Trainium2 NeuronCores (bf16).

Shards the 32 attention heads across 8 cores (4 heads each). All matmuls run
in bf16 (f32 PSUM accumulation); end-to-end max-rel error vs the f32
reference is ~5e-3 (gate is 2e-2).

Per-core structure:
  Phase 1  QKV projection, weight-stationary: out[o, tok] = sum_d w[d,o] x[d,tok].
           Q/K land directly in the [head-dim, token] layout attention needs
           (no PE transposes); RoPE is applied on PSUM evict using
           host-transposed cos/sin [64, T] (even head dims in partitions
           0-63, odd in 64-127 via a host-side row permutation of wq/wk).
           V is transposed to [token, e] via the DMA XBAR (16-bit transpose).
  Phase 2  Attention per (b, head): one key-tile sweep jt=0..15; per jt up to
           4 query-macro score matmuls share the stationary k block; causal
           mask is a vector add into PSUM; exp on the scalar engine writes
           bf16 prob tiles; softmax denominators accumulate on gpsimd
           (no PE ones-matmuls); AV matmuls trail one jt for pipelining.
  Phase 3  Output projection from SBUF-resident per-head ot tiles.

Each core computes its heads' contribution through wo; the host sums the 8
partial outputs.
"""

import math
import sys

sys.path.insert(0, "/opt/trn_rl_repo")

import numpy as np
import ml_dtypes

import concourse.bacc as bacc
import concourse.bass as bass
import concourse.mybir as mybir
import concourse.tile as tile

F32 = mybir.dt.float32
F32R = mybir.dt.float32r
BF16 = mybir.dt.bfloat16
AF = mybir.ActivationFunctionType
ALU = mybir.AluOpType

HEAD_DIM = 128
NEG = -1.0e30


class Cfg:
    def __init__(self, B=2, S=2048, D=4096, H_PER=4, n_cores=8):
        self.B, self.S, self.D, self.H_PER = B, S, D, H_PER
        self.n_cores = n_cores
        self.T = B * S                    # total tokens (batch-major)
        self.O = H_PER * HEAD_DIM         # per-core projection width
        self.DC = D // 128                # contraction chunks
        self.TT = self.T // 128           # 128-token tiles
        self.TM = self.T // 512           # 512-token macro tiles
        self.NJT = S // 128               # key tiles per batch


def build(cfg: Cfg) -> bacc.Bacc:
    B, S, D, T, O = cfg.B, cfg.S, cfg.D, cfg.T, cfg.O
    H_PER, DC, TT, TM, NJT = cfg.H_PER, cfg.DC, cfg.TT, cfg.TM, cfg.NJT
    scale = 1.0 / math.sqrt(HEAD_DIM)

    nc = bacc.Bacc(None, target_bir_lowering=False)

    xt = nc.dram_tensor("xt", [TM, 128, DC, 512], BF16, kind="ExternalInput")
    wqt = nc.dram_tensor("wqt", [128, DC, O], BF16, kind="ExternalInput")
    wkt = nc.dram_tensor("wkt", [128, DC, O], BF16, kind="ExternalInput")
    wvt = nc.dram_tensor("wvt", [128, DC, O], BF16, kind="ExternalInput")
    wot = nc.dram_tensor("wot", [128, H_PER, D], BF16, kind="ExternalInput")
    cost = nc.dram_tensor("cost", [64, T], BF16, kind="ExternalInput")
    sint = nc.dram_tensor("sint", [64, T], BF16, kind="ExternalInput")
    maskt = nc.dram_tensor("maskt", [4 * 128, 512], F32, kind="ExternalInput")
    onest = nc.dram_tensor("onest", [128, 128], F32R, kind="ExternalInput")
    out = nc.dram_tensor("out", [T, D], F32, kind="ExternalOutput")

    # DRAM scratch: q/k in [head-dim, token] layout; v per head [tok%128, jt, e]
    q_s = nc.dram_tensor("q_s", [O, T], BF16)
    k_s = nc.dram_tensor("k_s", [O, T], BF16)
    v_s = nc.dram_tensor("v_s", [H_PER, 128, B * NJT, 128], BF16)

    with tile.TileContext(nc) as tc:
        # ================= Phase 1: QKV projection (weight-stationary) =====
        with tc.tile_pool(name="w1", bufs=1) as wp, \
             tc.tile_pool(name="sb1", bufs=1) as sb, \
             tc.tile_pool(name="ps1", bufs=1, space="PSUM") as ps:
            cos_sb = wp.tile([64, T], BF16, name="cos_sb")
            sin_sb = wp.tile([64, T], BF16, name="sin_sb")
            wq_sb = wp.tile([128, DC, O], BF16, name="wq_sb")
            wk_sb = wp.tile([128, DC, O], BF16, name="wk_sb")
            wv_sb = wp.tile([128, DC, O], BF16, name="wv_sb")
            for wsb, wdram in ((wq_sb, wqt), (wk_sb, wkt), (wv_sb, wvt)):
                for c4 in range(4):
                    nc.scalar.dma_start(out=wsb[:, c4 * 8:(c4 + 1) * 8, :],
                                        in_=wdram[:, c4 * 8:(c4 + 1) * 8, :])

            for tm in range(TM):
                xq = [sb.tile([128, DC // 4, 512], BF16, name="xs", tag=f"xs{q}",
                              bufs=2) for q in range(4)]
                for q in range(4):
                    nc.sync.dma_start(out=xq[q][:],
                                      in_=xt[tm][:, q * 8:(q + 1) * 8, :])
                    if tm == 0 and q == 0:
                        nc.sync.dma_start(out=cos_sb[:], in_=cost[:])
                        nc.sync.dma_start(out=sin_sb[:], in_=sint[:])
                cs = cos_sb[:, tm * 512:(tm + 1) * 512]
                sn = sin_sb[:, tm * 512:(tm + 1) * 512]
                for ob in range(12):
                    wsb = (wq_sb, wk_sb, wv_sb)[ob // 4]
                    col = ob % 4
                    pp = ps.tile([128, 512], F32, name="pp", tag=f"p{ob % 8}",
                                 bufs=1)
                    for d in range(DC):
                        nc.tensor.matmul(pp[:], wsb[:, d, col * 128:(col + 1) * 128],
                                         xq[d // 8][:, d % 8, :], start=(d == 0),
                                         stop=(d == DC - 1))
                    if ob < 8:
                        # RoPE on evict: partitions 0-63 even dims, 64-127 odd
                        dst = q_s if ob < 4 else k_s
                        pe, po = pp[0:64, :], pp[64:128, :]
                        rot = sb.tile([128, 512], BF16, name="rot", tag="rot", bufs=3)
                        tec = sb.tile([64, 512], F32, name="tec", tag="tec", bufs=2)
                        tos = sb.tile([64, 512], F32, name="tos", tag="tos", bufs=2)
                        tes = sb.tile([64, 512], F32, name="tes", tag="tes", bufs=2)
                        toc = sb.tile([64, 512], F32, name="toc", tag="toc", bufs=2)
                        nc.vector.tensor_tensor(tec[:], pe, cs, ALU.mult)
                        nc.vector.tensor_tensor(tos[:], po, sn, ALU.mult)
                        nc.vector.tensor_tensor(tes[:], pe, sn, ALU.mult)
                        nc.vector.tensor_tensor(toc[:], po, cs, ALU.mult)
                        nc.gpsimd.tensor_tensor(rot[0:64, :], tec[:], tos[:],
                                                ALU.subtract)
                        nc.gpsimd.tensor_tensor(rot[64:128, :], tes[:], toc[:],
                                                ALU.add)
                        nc.sync.dma_start(
                            out=dst[col * 128:(col + 1) * 128,
                                    tm * 512:(tm + 1) * 512],
                            in_=rot[:])
                    else:
                        # V: evict bf16 then transpose 128x128 blocks via XBAR
                        vt = sb.tile([128, 512], BF16, name="vt", tag="vt", bufs=2)
                        nc.scalar.activation(vt[:], pp[:], AF.Copy)
                        vstg = sb.tile([128, 4, 128], BF16, name="vstg",
                                       tag="vstg", bufs=2)
                        for blk in range(4):
                            nc.scalar.dma_start(out=vstg[:, blk, :],
                                                in_=vt[:, blk * 128:(blk + 1) * 128],
                                                transpose=True)
                        nc.scalar.dma_start(out=v_s[col][:, tm * 4:tm * 4 + 4, :],
                                            in_=vstg[:])

        # ================= Phase 2+3: attention + output projection ========
        with tc.tile_pool(name="otp", bufs=1) as otp:
            ot_sb = [otp.tile([128, T], BF16, name=f"ot{h}") for h in range(H_PER)]
            wo_sb = otp.tile([128, H_PER, D], BF16, name="wo_sb")
            with tc.tile_pool(name="asb", bufs=1) as asb, \
                 tc.tile_pool(name="aps", bufs=1, space="PSUM") as aps, \
                 tc.tile_pool(name="sb3", bufs=1) as sb3, \
                 tc.tile_pool(name="ps3", bufs=1, space="PSUM") as ps3:
                mask_sb = asb.tile([128, 4, 512], F32, name="mask_sb")
                ones_sb = asb.tile([128, 128], F32R, name="ones_sb")
                nc.sync.dma_start(out=mask_sb[:],
                                  in_=maskt[:].rearrange("(m p) i -> p m i", p=128))
                nc.sync.dma_start(out=ones_sb[:], in_=onest[:])
                def load_kqv(h):
                    kt = asb.tile([128, T], BF16, name="kt", tag="kt", bufs=2)
                    qt = asb.tile([128, T], BF16, name="qt", tag="qt", bufs=2)
                    vv = asb.tile([128, B * NJT, 128], BF16, name="vv", tag="vv",
                                  bufs=2)
                    nc.sync.dma_start(out=kt[:], in_=k_s[h * 128:(h + 1) * 128, :])
                    nc.sync.dma_start(out=qt[:], in_=q_s[h * 128:(h + 1) * 128, :])
                    nc.sync.dma_start(out=vv[:], in_=v_s[h])
                    return kt, qt, vv

                def half_sweep(b, h, kt, qt, vv, im_lo):
                    ims_all = (im_lo, im_lo + 1)
                    njts = 4 * (im_lo + 2)
                    pot = {im: aps.tile([128, 512], F32, name="pot",
                                        tag=f"pot{im % 2}", bufs=1)
                           for im in ims_all}
                    ds = {im: asb.tile([128, 512], F32R, name="ds",
                                       tag=f"ds{im % 2}", bufs=2)
                          for im in ims_all}

                    def emit_av(jt, pts):
                        for im in ims_all:
                            if jt < 4 * (im + 1):
                                nc.tensor.matmul(pot[im][:],
                                                 vv[:, b * NJT + jt, :],
                                                 pts[im][:],
                                                 start=(jt == 0),
                                                 stop=(jt == 4 * im + 3))

                    prev = None
                    for jt in range(njts):
                        ims = [im for im in ims_all if jt < 4 * (im + 1)]
                        im_d = jt // 4
                        cur_pst = {}
                        for im in ims:
                            pp = aps.tile([128, 512], F32, name="pst",
                                          tag="pst", bufs=6)
                            nc.tensor.matmul(
                                pp[:],
                                kt[:, b * S + jt * 128:b * S + jt * 128 + 128],
                                qt[:, b * S + im * 512:b * S + (im + 1) * 512],
                                start=True, stop=True)
                            cur_pst[im] = pp
                        if im_d in ims:
                            nc.vector.tensor_tensor(cur_pst[im_d][:],
                                                    cur_pst[im_d][:],
                                                    mask_sb[:, jt % 4, :],
                                                    ALU.add)
                        if prev is not None:
                            emit_av(*prev)
                        cur_pt = {}
                        # exp the masked (diag) tile last: its vector-add dep
                        # shouldn't stall the scalar engine
                        order = [im for im in ims if im != im_d] + \
                                ([im_d] if im_d in ims else [])
                        for im in order:
                            pt_t = asb.tile([128, 512], BF16, name="pt",
                                            tag=f"pt{im % 2}", bufs=3)
                            nc.scalar.activation(pt_t[:], cur_pst[im][:], AF.Exp,
                                                 scale=scale)
                            eng = nc.gpsimd if im % 2 == 0 else nc.vector
                            if jt == 0:
                                eng.tensor_copy(ds[im][:], pt_t[:])
                            else:
                                eng.tensor_tensor(ds[im][:], ds[im][:], pt_t[:],
                                                  ALU.add)
                            cur_pt[im] = pt_t
                        prev = (jt, cur_pt)
                    emit_av(*prev)
                    for im in ims_all:
                        pden = aps.tile([128, 512], F32, name="pden",
                                        tag="pst", bufs=6)
                        nc.tensor.matmul(pden[:], ones_sb[:], ds[im][:],
                                         start=True, stop=True)
                        r = asb.tile([128, 512], F32, name="rs", tag="rs", bufs=2)
                        with nc.allow_low_precision(reason="softmax recip"):
                            nc.vector.reciprocal(r[:], pden[:])
                        nc.vector.tensor_tensor(
                            ot_sb[h][:, b * S + im * 512:b * S + (im + 1) * 512],
                            pot[im][:], r[:], ALU.mult)

                for h in range(H_PER):
                    kt, qt, vv = load_kqv(h)
                    nc.sync.dma_start(out=wo_sb[:, h, :], in_=wot[:, h, :])
                    half_sweep(0, h, kt, qt, vv, 0)
                    half_sweep(0, h, kt, qt, vv, 2)
                for h in range(H_PER):
                    kt, qt, vv = load_kqv(h)
                    half_sweep(1, h, kt, qt, vv, 0)
                    half_sweep(1, h, kt, qt, vv, 2)

            # ================= Phase 3: output projection ==================
            with tc.tile_pool(name="sb3", bufs=1) as sb3, \
                 tc.tile_pool(name="ps3", bufs=1, space="PSUM") as ps3:
                for tt in range(TT):
                    pf = [ps3.tile([128, 512], F32, name=f"pf{e}", tag=f"pf{e}",
                                   bufs=1) for e in range(8)]
                    for hh in range(H_PER):
                        for e in range(8):
                            nc.tensor.matmul(
                                pf[e][:], ot_sb[hh][:, tt * 128:(tt + 1) * 128],
                                wo_sb[:, hh, e * 512:(e + 1) * 512],
                                start=(hh == 0), stop=(hh == H_PER - 1))
                    f_sb = sb3.tile([128, D], F32, name="f_sb", tag="f_sb", bufs=3)
                    for e in range(8):
                        if e % 2 == 0:
                            nc.vector.tensor_copy(f_sb[:, e * 512:(e + 1) * 512],
                                                  pf[e][:])
                        else:
                            nc.scalar.activation(f_sb[:, e * 512:(e + 1) * 512],
                                                 pf[e][:], AF.Copy)
                    q = nc.sync if tt % 2 == 0 else nc.scalar
                    q.dma_start(out=out[tt * 128:(tt + 1) * 128, :], in_=f_sb[:])

    nc.compile()
    return nc


# host-side even/odd permutation of head dims (RoPE becomes partition-half
# elementwise in the [head-dim, token] layout)
_PERM = np.concatenate([np.arange(0, HEAD_DIM, 2), np.arange(1, HEAD_DIM, 2)])


def host_inputs(cfg: Cfg, x, wq, wk, wv, wo, freqs_cos, freqs_sin):
    """Build the 8 per-core input maps from full inputs (numpy f32)."""
    B, S, D, T, O, H_PER = cfg.B, cfg.S, cfg.D, cfg.T, cfg.O, cfg.H_PER
    DC, TM = cfg.DC, cfg.TM
    bf = ml_dtypes.bfloat16

    xb = np.ascontiguousarray(x.reshape(T, D)).astype(bf)
    # xt[tm, p, c, t] = x.T[c*128+p, tm*512+t]
    xt = np.ascontiguousarray(
        xb.T.reshape(DC, 128, TM, 512).transpose(2, 1, 0, 3))

    cosT = np.ascontiguousarray(np.tile(freqs_cos, (B, 1)).T).astype(bf)
    sinT = np.ascontiguousarray(np.tile(freqs_sin, (B, 1)).T).astype(bf)

    # maskt[k*128 + j, i] = 0 if k*128 + j <= i else NEG
    j_idx = np.arange(4 * 128)[:, None]
    i_idx = np.arange(512)[None, :]
    maskt = np.where(j_idx <= i_idx, 0.0, NEG).astype(np.float32)

    def wtile(w_rows):  # [D, O] -> [128, DC, O]
        return np.ascontiguousarray(
            w_rows.T.reshape(DC, 128, O).transpose(1, 0, 2)).astype(bf)

    in_maps = []
    for c in range(cfg.n_cores):
        rows = []
        for hh in range(H_PER):
            base = (c * H_PER + hh) * HEAD_DIM
            rows.append(base + _PERM)
        prows = np.concatenate(rows)                     # permuted rows for q/k
        nrows = np.arange(c * O, (c + 1) * O)            # natural rows for v
        wot_t = np.ascontiguousarray(
            wo[:, nrows].T.reshape(H_PER, 128, D).transpose(1, 0, 2)).astype(bf)
        in_maps.append({
            "xt": xt,
            "wqt": wtile(wq[prows]),
            "wkt": wtile(wk[prows]),
            "wvt": wtile(wv[nrows]),
            "wot": wot_t,
            "cost": cosT, "sint": sinT, "maskt": maskt,
            "onest": np.ones((128, 128), np.float32),
        })
    return in_maps


_CACHE = {}


def kernel(x, wq, wk, wv, wo, freqs_cos, freqs_sin, mask=None, start_pos=0):
    cfg = Cfg()
    x = np.asarray(x, dtype=np.float32)
    in_maps = host_inputs(cfg, x, np.asarray(wq, np.float32),
                          np.asarray(wk, np.float32),
                          np.asarray(wv, np.float32),
                          np.asarray(wo, np.float32),
                          np.asarray(freqs_cos, np.float32),
                          np.asarray(freqs_sin, np.float32))
    if "nc" not in _CACHE:
        _CACHE["nc"] = build(cfg)
    from concourse.bass_utils import run_bass_kernel_spmd
    res = run_bass_kernel_spmd(_CACHE["nc"], in_maps, core_ids=list(range(cfg.n_cores)))
    acc = res.results[0]["out"].astype(np.float64)
    for c in range(1, cfg.n_cores):
        acc = acc + res.results[c]["out"]
    return acc.astype(np.float32).reshape(cfg.B, cfg.S, cfg.D)
